# revision 14
# baseline (speedup 1.0000x reference)
"""Trainium2 Bass kernel for nn_Gate_Net (sigmoid gate cumprod over doc windows).

Math per doc (L=128 sentences-1, K=127 window offsets), scores s[129]:
  f = s[:128], b = s[1:129]
  fwd_gate[j,k] = sigmoid(100*(f[j-k] - f[j]) + 5)   (f[j-k]=0 if j<k)
  bwd_gate[j,k] = sigmoid(100*(b[j+k+1] - b[j]) + 5) (b[j+k+1]=0 if j+k+1>=128)
  out = stack([cumprod_k fwd_gate, cumprod_k bwd_gate])  -> [2, N, 128, 127]

Device strategy (per core, 256 docs in 2 blocks of 128):
  - gather docs = score[idx] via indirect DMA (docs on partitions)
  - PE-transpose -> F[t,d] / B[t,d]; split fp32 -> bf16 hi/mid/lo (exact to ~2^-27)
  - arg[d,(j,k)] = sum_p F[p,d] * W[p,(j,k)] with host-built constant
      W_fwd[p,(j,k)] = [p==j-k] - [p==j],  W_bwd[p,(j,k)] = [p==j+k+1] - [p==j]
    as 3 accumulating bf16 matmuls (stationary = F split, moving = W slice)
  - ACT: gate = sigmoid(100*psum + 5), PSUM->SBUF
  - DVE tensor_tensor_scan(op0=max, data0=mask(1.0 at k==0), op1=mult, data1=gate)
    = segmented cumprod (gates <= 1 so max(1,state) resets exactly)
  - stripe DMA out: [d, (j,k)] rows are contiguous per doc in HBM
"""

import sys

sys.path.insert(0, "/opt/trn_rl_repo")

import numpy as np
import ml_dtypes

import concourse.bacc as bacc
import concourse.bass as bass
import concourse.tile as tile
from concourse import mybir
from concourse.bass_utils import run_bass_kernel_spmd

N_CORES = 8
POOL = 300000
N_DOCS = 2048
DOC_LEN = 129
L = DOC_LEN - 1          # 128
K = L - 1                # 127
JK = L * K               # 16256 flattened (j,k)
DOCS_PER_CORE = N_DOCS // N_CORES  # 256
BLOCKS = DOCS_PER_CORE // 128      # 2
STRIPE = 2048
N_STRIPES = (JK + STRIPE - 1) // STRIPE  # 8 (last = 1920)
MM_WIN = 512

_BF16 = ml_dtypes.bfloat16


def _build_consts():
    j = np.arange(L)[:, None]   # [128,1]
    k = np.arange(K)[None, :]   # [1,127]
    p = np.arange(128)[:, None, None]
    w_fwd = ((j[None] - k[None]) == p).astype(np.float32) - (
        (j[None] == p) & np.ones_like(k[None], bool)
    ).astype(np.float32)
    w_bwd = ((j[None] + k[None] + 1) == p).astype(np.float32) - (
        (j[None] == p) & np.ones_like(k[None], bool)
    ).astype(np.float32)
    w_fwd = w_fwd.reshape(128, JK).astype(_BF16)
    w_bwd = w_bwd.reshape(128, JK).astype(_BF16)
    ident = np.eye(128, dtype=np.float32)
    smask = np.zeros((128, STRIPE + 128), np.float32)
    smask[:, ::K] = 1.0
    return w_fwd, w_bwd, ident, smask


def build_program():
    nc = bacc.Bacc("TRN2", target_bir_lowering=False, debug=False,
                   num_swdge_queues=2)
    f32 = mybir.dt.float32
    bf16 = mybir.dt.bfloat16

    score_d = nc.dram_tensor("score", [POOL, 1], f32, kind="ExternalInput")
    idx_d = nc.dram_tensor("idx", [DOCS_PER_CORE, DOC_LEN], mybir.dt.int32,
                           kind="ExternalInput")
    wf_d = nc.dram_tensor("w_fwd", [128, JK], bf16, kind="ExternalInput")
    wb_d = nc.dram_tensor("w_bwd", [128, JK], bf16, kind="ExternalInput")
    id_d = nc.dram_tensor("ident", [128, 128], f32, kind="ExternalInput")
    sm_d = nc.dram_tensor("smask", [128, STRIPE + 128], f32,
                          kind="ExternalInput")
    out_d = nc.dram_tensor("out", [2, DOCS_PER_CORE, JK], bf16,
                           kind="ExternalOutput")

    with tile.TileContext(nc) as tc:
        with (
            tc.tile_pool(name="consts", bufs=1) as consts,
            tc.tile_pool(name="prep", bufs=6) as prep,
            tc.tile_pool(name="gates", bufs=2) as gates,
            tc.tile_pool(name="outs", bufs=2) as outs,
            tc.tile_pool(name="outs16", bufs=18) as outs16,
            tc.tile_pool(name="psum", bufs=2, space="PSUM") as psum,
        ):
            # ---- constants ----
            ident = consts.tile([128, 128], f32)
            nc.scalar.dma_start(ident[:], id_d[:])

            w_sb = {}
            for dname, dram in (("f", wf_d), ("b", wb_d)):
                wt = consts.tile([128, JK], bf16, tag=f"w_{dname}")
                # chunked load so early stripes only depend on their chunk
                for c0 in range(0, JK, 4064):
                    cl = min(4064, JK - c0)
                    nc.scalar.dma_start(wt[:, c0:c0 + cl], dram[:, c0:c0 + cl])
                w_sb[dname] = wt

            # mask master: 1.0 where (col % 127)==0 (host-built)
            mask = consts.tile([128, STRIPE + 128], f32)
            nc.scalar.dma_start(mask[:], sm_d[:])

            bias5 = consts.tile([128, 1], f32)
            nc.gpsimd.memset(bias5[:], 5.0)

            # ---- per block: gather + prep + its two (f,b) groups ----
            deferred = []
            for blk in range(BLOCKS):
                idx_sb = consts.tile([128, DOC_LEN], mybir.dt.int32,
                                     tag=f"idx{blk}")
                nc.sync.dma_start(idx_sb[:], idx_d[blk * 128:(blk + 1) * 128, :])
                docs = consts.tile([128, DOC_LEN], f32, tag=f"docs{blk}")
                # HW indirect DMA consumes ONE index per dest partition-row:
                # gather one column (128 arbitrary elements) per instruction.
                for t in range(DOC_LEN):
                    gi_ = nc.gpsimd.indirect_dma_start(
                        out=docs[:, t:t + 1],
                        out_offset=None,
                        in_=score_d[:],
                        in_offset=bass.IndirectOffsetOnAxis(
                            ap=idx_sb[:, t:t + 1], axis=0),
                    )
                    if t % 2 == 1:
                        gi_.ins.queue = "qPoolDynamic1"
                if blk == 1:
                    # blk1 gathers issued: drain blk0's buffered output
                    # stripes from the gpsimd queue AFTER the gathers so
                    # their DMA-engine traffic cannot precede them
                    for (odst, osrc) in deferred:
                        nc.gpsimd.dma_start(odst, osrc)
                    deferred = []
                splits = {}
                for dname, off in (("f", 0), ("b", 1)):
                    ps = psum.tile([128, STRIPE], f32, tag="mm")
                    tps = ps[:, 0:128]
                    nc.tensor.transpose(tps, docs[:, off:off + 128], ident[:])
                    hi = consts.tile([128, 128], bf16, tag=f"hi{blk}{dname}")
                    nc.vector.tensor_copy(hi[:], tps)
                    hi32 = prep.tile([128, 128], f32, tag="t32")
                    nc.vector.tensor_copy(hi32[:], hi[:])
                    t1 = prep.tile([128, 128], f32, tag="t32")
                    nc.vector.tensor_sub(t1[:], tps, hi32[:])
                    mid = consts.tile([128, 128], bf16, tag=f"mid{blk}{dname}")
                    nc.vector.tensor_copy(mid[:], t1[:])
                    splits[(blk, dname)] = [hi, mid]

                for di, dname in enumerate(("f", "b")):
                    wt = w_sb[dname]
                    sp = splits[(blk, dname)]
                    prev_out = None
                    prev_len = 0
                    stripes = []
                    _c = 0
                    first = 512 if (blk == 1 and di == 0) else STRIPE
                    while _c < JK:
                        _l = min(first if _c == 0 else STRIPE, JK - _c)
                        if blk == 1 and di == 1 and _c + _l >= JK and _l > 1024:
                            q4 = _l // 4
                            for _i in range(3):
                                stripes.append((_c + _i * q4, q4))
                            stripes.append((_c + 3 * q4, _l - 3 * q4))
                        else:
                            stripes.append((_c, _l))
                        _c += _l
                    for s, (c0, ln) in enumerate(stripes):
                        ps = psum.tile([128, STRIPE], f32, tag="mm")
                        for w0 in range(0, ln, MM_WIN):
                            wl = min(MM_WIN, ln - w0)
                            for si in range(len(sp)):
                                nc.tensor.matmul(
                                    ps[:, w0:w0 + wl],
                                    sp[si][:],
                                    wt[:, c0 + w0:c0 + w0 + wl],
                                    start=(si == 0),
                                    stop=(si == len(sp) - 1),
                                )
                        gate = gates.tile([128, STRIPE], f32)
                        nc.scalar.activation(
                            gate[:, :ln], ps[:, :ln],
                            mybir.ActivationFunctionType.Sigmoid,
                            bias=bias5[:], scale=100.0,
                        )
                        ot = outs.tile([128, STRIPE], f32)
                        q = c0 % K
                        init = 0.0 if s == 0 else prev_out[:, prev_len - 1:prev_len]
                        nc.vector.tensor_tensor_scan(
                            out=ot[:, :ln],
                            data0=mask[:, q:q + ln],
                            data1=gate[:, :ln],
                            initial=init,
                            op0=mybir.AluOpType.max,
                            op1=mybir.AluOpType.mult,
                        )
                        ot16 = outs16.tile([128, STRIPE], bf16)
                        nc.scalar.copy(ot16[:, :ln], ot[:, :ln])
                        if blk == 0:
                            deferred.append(
                                (out_d[di, blk * 128:(blk + 1) * 128,
                                       c0:c0 + ln], ot16[:, :ln]))
                        else:
                            nc.sync.dma_start(
                                out_d[di, blk * 128:(blk + 1) * 128,
                                      c0:c0 + ln],
                                ot16[:, :ln],
                            )
                        prev_out, prev_len = ot, ln

    nc.compile()
    return nc


_NC = None


def _get_nc():
    global _NC
    if _NC is None:
        _NC = build_program()
    return _NC


def kernel(score, score_idx):
    score = np.ascontiguousarray(np.asarray(score, dtype=np.float32))
    idx = np.ascontiguousarray(np.asarray(score_idx).astype(np.int32))
    assert score.shape == (POOL,) and idx.shape == (N_DOCS, DOC_LEN)

    w_fwd, w_bwd, ident, smask = _build_consts()
    nc = _get_nc()

    in_maps = []
    for c in range(N_CORES):
        in_maps.append({
            "score": score.reshape(POOL, 1),
            "idx": idx[c * DOCS_PER_CORE:(c + 1) * DOCS_PER_CORE],
            "w_fwd": w_fwd,
            "w_bwd": w_bwd,
            "ident": ident,
            "smask": smask,
        })
    res = run_bass_kernel_spmd(nc, in_maps, core_ids=list(range(N_CORES)))
    shards = [np.asarray(r["out"]).astype(np.float32).reshape(2, DOCS_PER_CORE, L, K)
              for r in res.results]
    return np.concatenate(shards, axis=1)


if __name__ == "__main__":
    rng = np.random.default_rng(0)
    score = rng.standard_normal(POOL).astype(np.float32)
    idx = rng.integers(0, POOL, size=(N_DOCS, DOC_LEN)).astype(np.int32)
    out = kernel(score, idx)
    print(out.shape, out.dtype, float(out[0, 0, :4, :4].sum()))



# revision 15
# speedup vs baseline: 1.0123x; 1.0123x over previous
"""Trainium2 Bass kernel for nn_Gate_Net (sigmoid gate cumprod over doc windows).

Math per doc (L=128 sentences-1, K=127 window offsets), scores s[129]:
  f = s[:128], b = s[1:129]
  fwd_gate[j,k] = sigmoid(100*(f[j-k] - f[j]) + 5)   (f[j-k]=0 if j<k)
  bwd_gate[j,k] = sigmoid(100*(b[j+k+1] - b[j]) + 5) (b[j+k+1]=0 if j+k+1>=128)
  out = stack([cumprod_k fwd_gate, cumprod_k bwd_gate])  -> [2, N, 128, 127]

Device strategy (per core, 256 docs in 2 blocks of 128):
  - gather docs = score[idx] via indirect DMA (docs on partitions)
  - PE-transpose -> F[t,d] / B[t,d]; split fp32 -> bf16 hi/mid/lo (exact to ~2^-27)
  - arg[d,(j,k)] = sum_p F[p,d] * W[p,(j,k)] with host-built constant
      W_fwd[p,(j,k)] = [p==j-k] - [p==j],  W_bwd[p,(j,k)] = [p==j+k+1] - [p==j]
    as 3 accumulating bf16 matmuls (stationary = F split, moving = W slice)
  - ACT: gate = sigmoid(100*psum + 5), PSUM->SBUF
  - DVE tensor_tensor_scan(op0=max, data0=mask(1.0 at k==0), op1=mult, data1=gate)
    = segmented cumprod (gates <= 1 so max(1,state) resets exactly)
  - stripe DMA out: [d, (j,k)] rows are contiguous per doc in HBM
"""

import sys

sys.path.insert(0, "/opt/trn_rl_repo")

import numpy as np
import ml_dtypes

import concourse.bacc as bacc
import concourse.bass as bass
import concourse.tile as tile
from concourse import mybir
from concourse.bass_utils import run_bass_kernel_spmd

N_CORES = 8
POOL = 300000
N_DOCS = 2048
DOC_LEN = 129
L = DOC_LEN - 1          # 128
K = L - 1                # 127
JK = L * K               # 16256 flattened (j,k)
DOCS_PER_CORE = N_DOCS // N_CORES  # 256
BLOCKS = DOCS_PER_CORE // 128      # 2
STRIPE = 2048
N_STRIPES = (JK + STRIPE - 1) // STRIPE  # 8 (last = 1920)
MM_WIN = 512

_BF16 = ml_dtypes.bfloat16


def _build_consts():
    j = np.arange(L)[:, None]   # [128,1]
    k = np.arange(K)[None, :]   # [1,127]
    p = np.arange(128)[:, None, None]
    w_fwd = ((j[None] - k[None]) == p).astype(np.float32) - (
        (j[None] == p) & np.ones_like(k[None], bool)
    ).astype(np.float32)
    w_bwd = ((j[None] + k[None] + 1) == p).astype(np.float32) - (
        (j[None] == p) & np.ones_like(k[None], bool)
    ).astype(np.float32)
    w_fwd = w_fwd.reshape(128, JK).astype(_BF16)
    w_bwd = w_bwd.reshape(128, JK).astype(_BF16)
    ident = np.eye(128, dtype=np.float32)
    smask = np.zeros((128, STRIPE + 128), np.float32)
    smask[:, ::K] = 1.0
    return w_fwd, w_bwd, ident, smask


def build_program():
    nc = bacc.Bacc("TRN2", target_bir_lowering=False, debug=False,
                   num_swdge_queues=2)
    f32 = mybir.dt.float32
    bf16 = mybir.dt.bfloat16

    score_d = nc.dram_tensor("score", [POOL, 1], f32, kind="ExternalInput")
    idx_d = nc.dram_tensor("idx", [DOCS_PER_CORE, DOC_LEN], mybir.dt.int32,
                           kind="ExternalInput")
    wf_d = nc.dram_tensor("w_fwd", [128, JK], bf16, kind="ExternalInput")
    wb_d = nc.dram_tensor("w_bwd", [128, JK], bf16, kind="ExternalInput")
    id_d = nc.dram_tensor("ident", [128, 128], f32, kind="ExternalInput")
    sm_d = nc.dram_tensor("smask", [128, STRIPE + 128], f32,
                          kind="ExternalInput")
    out_d = nc.dram_tensor("out", [2, DOCS_PER_CORE, JK], bf16,
                           kind="ExternalOutput")

    with tile.TileContext(nc) as tc:
        with (
            tc.tile_pool(name="consts", bufs=1) as consts,
            tc.tile_pool(name="prep", bufs=6) as prep,
            tc.tile_pool(name="gates", bufs=2) as gates,
            tc.tile_pool(name="outs", bufs=2) as outs,
            tc.tile_pool(name="outs16", bufs=18) as outs16,
            tc.tile_pool(name="psum", bufs=2, space="PSUM") as psum,
        ):
            # ---- constants ----
            ident = consts.tile([128, 128], f32)
            nc.scalar.dma_start(ident[:], id_d[:])

            w_sb = {}
            for dname, dram in (("f", wf_d), ("b", wb_d)):
                wt = consts.tile([128, JK], bf16, tag=f"w_{dname}")
                # chunked load so early stripes only depend on their chunk
                for c0 in range(0, JK, 4064):
                    cl = min(4064, JK - c0)
                    nc.scalar.dma_start(wt[:, c0:c0 + cl], dram[:, c0:c0 + cl])
                w_sb[dname] = wt

            # mask master: 1.0 where (col % 127)==0 (host-built)
            mask = consts.tile([128, STRIPE + 128], f32)
            nc.scalar.dma_start(mask[:], sm_d[:])

            bias5 = consts.tile([128, 1], f32)
            nc.gpsimd.memset(bias5[:], 5.0)

            # ---- per block: gather + prep + its two (f,b) groups ----
            deferred = []
            for blk in range(BLOCKS):
                idx_sb = consts.tile([128, DOC_LEN], mybir.dt.int32,
                                     tag=f"idx{blk}")
                nc.sync.dma_start(idx_sb[:], idx_d[blk * 128:(blk + 1) * 128, :])
                docs = consts.tile([128, DOC_LEN], f32, tag=f"docs{blk}")
                # HW indirect DMA consumes ONE index per dest partition-row:
                # gather one column (128 arbitrary elements) per instruction.
                for t in range(DOC_LEN):
                    gi_ = nc.gpsimd.indirect_dma_start(
                        out=docs[:, t:t + 1],
                        out_offset=None,
                        in_=score_d[:],
                        in_offset=bass.IndirectOffsetOnAxis(
                            ap=idx_sb[:, t:t + 1], axis=0),
                    )
                    if t % 2 == 1:
                        gi_.ins.queue = "qPoolDynamic1"
                if blk == 1:
                    # blk1 gathers issued: drain blk0's buffered output
                    # stripes from the gpsimd queue AFTER the gathers so
                    # their DMA-engine traffic cannot precede them
                    for (odst, osrc) in deferred:
                        nc.gpsimd.dma_start(odst, osrc)
                    deferred = []
                splits = {}
                for dname, off in (("f", 0), ("b", 1)):
                    ps = psum.tile([128, STRIPE], f32, tag="mm")
                    tps = ps[:, 0:128]
                    nc.tensor.transpose(tps, docs[:, off:off + 128], ident[:])
                    hi = consts.tile([128, 128], bf16, tag=f"hi{blk}{dname}")
                    nc.scalar.copy(hi[:], tps)
                    hi32 = prep.tile([128, 128], f32, tag="t32")
                    nc.vector.tensor_copy(hi32[:], hi[:])
                    t1 = prep.tile([128, 128], f32, tag="t32")
                    nc.vector.tensor_sub(t1[:], tps, hi32[:])
                    mid = consts.tile([128, 128], bf16, tag=f"mid{blk}{dname}")
                    nc.vector.tensor_copy(mid[:], t1[:])
                    splits[(blk, dname)] = [hi, mid]

                for di, dname in enumerate(("f", "b")):
                    wt = w_sb[dname]
                    sp = splits[(blk, dname)]
                    prev_out = None
                    prev_len = 0
                    stripes = []
                    _c = 0
                    while _c < JK:
                        _l = min(STRIPE, JK - _c)
                        if blk == 1 and di == 1 and _c + _l >= JK and _l > 1024:
                            stripes.append((_c, _l // 2))
                            stripes.append((_c + _l // 2, _l - _l // 2))
                        else:
                            stripes.append((_c, _l))
                        _c += _l
                    for s, (c0, ln) in enumerate(stripes):
                        ps = psum.tile([128, STRIPE], f32, tag="mm")
                        for w0 in range(0, ln, MM_WIN):
                            wl = min(MM_WIN, ln - w0)
                            for si in range(len(sp)):
                                nc.tensor.matmul(
                                    ps[:, w0:w0 + wl],
                                    sp[si][:],
                                    wt[:, c0 + w0:c0 + w0 + wl],
                                    start=(si == 0),
                                    stop=(si == len(sp) - 1),
                                )
                        gate = gates.tile([128, STRIPE], f32)
                        nc.scalar.activation(
                            gate[:, :ln], ps[:, :ln],
                            mybir.ActivationFunctionType.Sigmoid,
                            bias=bias5[:], scale=100.0,
                        )
                        ot = outs.tile([128, STRIPE], f32)
                        q = c0 % K
                        init = 0.0 if s == 0 else prev_out[:, prev_len - 1:prev_len]
                        nc.vector.tensor_tensor_scan(
                            out=ot[:, :ln],
                            data0=mask[:, q:q + ln],
                            data1=gate[:, :ln],
                            initial=init,
                            op0=mybir.AluOpType.max,
                            op1=mybir.AluOpType.mult,
                        )
                        ot16 = outs16.tile([128, STRIPE], bf16)
                        nc.scalar.copy(ot16[:, :ln], ot[:, :ln])
                        if blk == 0:
                            deferred.append(
                                (out_d[di, blk * 128:(blk + 1) * 128,
                                       c0:c0 + ln], ot16[:, :ln]))
                        else:
                            nc.sync.dma_start(
                                out_d[di, blk * 128:(blk + 1) * 128,
                                      c0:c0 + ln],
                                ot16[:, :ln],
                            )
                        prev_out, prev_len = ot, ln

    nc.compile()
    return nc


_NC = None


def _get_nc():
    global _NC
    if _NC is None:
        _NC = build_program()
    return _NC


def kernel(score, score_idx):
    score = np.ascontiguousarray(np.asarray(score, dtype=np.float32))
    idx = np.ascontiguousarray(np.asarray(score_idx).astype(np.int32))
    assert score.shape == (POOL,) and idx.shape == (N_DOCS, DOC_LEN)

    w_fwd, w_bwd, ident, smask = _build_consts()
    nc = _get_nc()

    in_maps = []
    for c in range(N_CORES):
        in_maps.append({
            "score": score.reshape(POOL, 1),
            "idx": idx[c * DOCS_PER_CORE:(c + 1) * DOCS_PER_CORE],
            "w_fwd": w_fwd,
            "w_bwd": w_bwd,
            "ident": ident,
            "smask": smask,
        })
    res = run_bass_kernel_spmd(nc, in_maps, core_ids=list(range(N_CORES)))
    shards = [np.asarray(r["out"]).astype(np.float32).reshape(2, DOCS_PER_CORE, L, K)
              for r in res.results]
    return np.concatenate(shards, axis=1)


if __name__ == "__main__":
    rng = np.random.default_rng(0)
    score = rng.standard_normal(POOL).astype(np.float32)
    idx = rng.integers(0, POOL, size=(N_DOCS, DOC_LEN)).astype(np.int32)
    out = kernel(score, idx)
    print(out.shape, out.dtype, float(out[0, 0, :4, :4].sum()))



# revision 16
# speedup vs baseline: 1.0123x; 1.0001x over previous
"""Trainium2 Bass kernel for nn_Gate_Net (sigmoid gate cumprod over doc windows).

Math per doc (L=128 sentences-1, K=127 window offsets), scores s[129]:
  f = s[:128], b = s[1:129]
  fwd_gate[j,k] = sigmoid(100*(f[j-k] - f[j]) + 5)   (f[j-k]=0 if j<k)
  bwd_gate[j,k] = sigmoid(100*(b[j+k+1] - b[j]) + 5) (b[j+k+1]=0 if j+k+1>=128)
  out = stack([cumprod_k fwd_gate, cumprod_k bwd_gate])  -> [2, N, 128, 127]

Device strategy (per core, 256 docs in 2 blocks of 128):
  - gather docs = score[idx] via indirect DMA (docs on partitions)
  - PE-transpose -> F[t,d] / B[t,d]; split fp32 -> bf16 hi/mid/lo (exact to ~2^-27)
  - arg[d,(j,k)] = sum_p F[p,d] * W[p,(j,k)] with host-built constant
      W_fwd[p,(j,k)] = [p==j-k] - [p==j],  W_bwd[p,(j,k)] = [p==j+k+1] - [p==j]
    as 3 accumulating bf16 matmuls (stationary = F split, moving = W slice)
  - ACT: gate = sigmoid(100*psum + 5), PSUM->SBUF
  - DVE tensor_tensor_scan(op0=max, data0=mask(1.0 at k==0), op1=mult, data1=gate)
    = segmented cumprod (gates <= 1 so max(1,state) resets exactly)
  - stripe DMA out: [d, (j,k)] rows are contiguous per doc in HBM
"""

import sys

sys.path.insert(0, "/opt/trn_rl_repo")

import numpy as np
import ml_dtypes

import concourse.bacc as bacc
import concourse.bass as bass
import concourse.tile as tile
from concourse import mybir
from concourse.bass_utils import run_bass_kernel_spmd

N_CORES = 8
POOL = 300000
N_DOCS = 2048
DOC_LEN = 129
L = DOC_LEN - 1          # 128
K = L - 1                # 127
JK = L * K               # 16256 flattened (j,k)
DOCS_PER_CORE = N_DOCS // N_CORES  # 256
BLOCKS = DOCS_PER_CORE // 128      # 2
STRIPE = 2048
N_STRIPES = (JK + STRIPE - 1) // STRIPE  # 8 (last = 1920)
MM_WIN = 512

_BF16 = ml_dtypes.bfloat16


def _build_consts():
    j = np.arange(L)[:, None]   # [128,1]
    k = np.arange(K)[None, :]   # [1,127]
    p = np.arange(128)[:, None, None]
    w_fwd = ((j[None] - k[None]) == p).astype(np.float32) - (
        (j[None] == p) & np.ones_like(k[None], bool)
    ).astype(np.float32)
    w_bwd = ((j[None] + k[None] + 1) == p).astype(np.float32) - (
        (j[None] == p) & np.ones_like(k[None], bool)
    ).astype(np.float32)
    w_fwd = w_fwd.reshape(128, JK).astype(_BF16)
    w_bwd = w_bwd.reshape(128, JK).astype(_BF16)
    ident = np.eye(128, dtype=np.float32)
    smask = np.zeros((128, STRIPE + 128), np.float32)
    smask[:, ::K] = 1.0
    return w_fwd, w_bwd, ident, smask


def build_program():
    nc = bacc.Bacc("TRN2", target_bir_lowering=False, debug=False,
                   num_swdge_queues=4)
    f32 = mybir.dt.float32
    bf16 = mybir.dt.bfloat16

    score_d = nc.dram_tensor("score", [POOL, 1], f32, kind="ExternalInput")
    idx_d = nc.dram_tensor("idx", [DOCS_PER_CORE, DOC_LEN], mybir.dt.int32,
                           kind="ExternalInput")
    wf_d = nc.dram_tensor("w_fwd", [128, JK], bf16, kind="ExternalInput")
    wb_d = nc.dram_tensor("w_bwd", [128, JK], bf16, kind="ExternalInput")
    id_d = nc.dram_tensor("ident", [128, 128], f32, kind="ExternalInput")
    sm_d = nc.dram_tensor("smask", [128, STRIPE + 128], f32,
                          kind="ExternalInput")
    out_d = nc.dram_tensor("out", [2, DOCS_PER_CORE, JK], bf16,
                           kind="ExternalOutput")

    with tile.TileContext(nc) as tc:
        with (
            tc.tile_pool(name="consts", bufs=1) as consts,
            tc.tile_pool(name="prep", bufs=6) as prep,
            tc.tile_pool(name="gates", bufs=2) as gates,
            tc.tile_pool(name="outs", bufs=2) as outs,
            tc.tile_pool(name="outs16", bufs=18) as outs16,
            tc.tile_pool(name="psum", bufs=2, space="PSUM") as psum,
        ):
            # ---- constants ----
            ident = consts.tile([128, 128], f32)
            nc.scalar.dma_start(ident[:], id_d[:])

            w_sb = {}
            for dname, dram in (("f", wf_d), ("b", wb_d)):
                wt = consts.tile([128, JK], bf16, tag=f"w_{dname}")
                # chunked load so early stripes only depend on their chunk
                for c0 in range(0, JK, 4064):
                    cl = min(4064, JK - c0)
                    nc.scalar.dma_start(wt[:, c0:c0 + cl], dram[:, c0:c0 + cl])
                w_sb[dname] = wt

            # mask master: 1.0 where (col % 127)==0 (host-built)
            mask = consts.tile([128, STRIPE + 128], f32)
            nc.scalar.dma_start(mask[:], sm_d[:])

            bias5 = consts.tile([128, 1], f32)
            nc.gpsimd.memset(bias5[:], 5.0)

            # ---- per block: gather + prep + its two (f,b) groups ----
            deferred = []
            for blk in range(BLOCKS):
                idx_sb = consts.tile([128, DOC_LEN], mybir.dt.int32,
                                     tag=f"idx{blk}")
                nc.sync.dma_start(idx_sb[:], idx_d[blk * 128:(blk + 1) * 128, :])
                docs = consts.tile([128, DOC_LEN], f32, tag=f"docs{blk}")
                # HW indirect DMA consumes ONE index per dest partition-row:
                # gather one column (128 arbitrary elements) per instruction.
                for t in range(DOC_LEN):
                    gi_ = nc.gpsimd.indirect_dma_start(
                        out=docs[:, t:t + 1],
                        out_offset=None,
                        in_=score_d[:],
                        in_offset=bass.IndirectOffsetOnAxis(
                            ap=idx_sb[:, t:t + 1], axis=0),
                    )
                    if t % 4:
                        gi_.ins.queue = f"qPoolDynamic{t % 4}"
                if blk == 1:
                    # blk1 gathers issued: drain blk0's buffered output
                    # stripes from the gpsimd queue AFTER the gathers so
                    # their DMA-engine traffic cannot precede them
                    for (odst, osrc) in deferred:
                        nc.gpsimd.dma_start(odst, osrc)
                    deferred = []
                splits = {}
                for dname, off in (("f", 0), ("b", 1)):
                    ps = psum.tile([128, STRIPE], f32, tag="mm")
                    tps = ps[:, 0:128]
                    nc.tensor.transpose(tps, docs[:, off:off + 128], ident[:])
                    hi = consts.tile([128, 128], bf16, tag=f"hi{blk}{dname}")
                    nc.scalar.copy(hi[:], tps)
                    hi32 = prep.tile([128, 128], f32, tag="t32")
                    nc.vector.tensor_copy(hi32[:], hi[:])
                    t1 = prep.tile([128, 128], f32, tag="t32")
                    nc.vector.tensor_sub(t1[:], tps, hi32[:])
                    mid = consts.tile([128, 128], bf16, tag=f"mid{blk}{dname}")
                    nc.vector.tensor_copy(mid[:], t1[:])
                    splits[(blk, dname)] = [hi, mid]

                for di, dname in enumerate(("f", "b")):
                    wt = w_sb[dname]
                    sp = splits[(blk, dname)]
                    prev_out = None
                    prev_len = 0
                    stripes = []
                    _c = 0
                    while _c < JK:
                        _l = min(STRIPE, JK - _c)
                        if blk == 1 and di == 1 and _c + _l >= JK and _l > 1024:
                            stripes.append((_c, _l // 2))
                            stripes.append((_c + _l // 2, _l - _l // 2))
                        else:
                            stripes.append((_c, _l))
                        _c += _l
                    for s, (c0, ln) in enumerate(stripes):
                        ps = psum.tile([128, STRIPE], f32, tag="mm")
                        for w0 in range(0, ln, MM_WIN):
                            wl = min(MM_WIN, ln - w0)
                            for si in range(len(sp)):
                                nc.tensor.matmul(
                                    ps[:, w0:w0 + wl],
                                    sp[si][:],
                                    wt[:, c0 + w0:c0 + w0 + wl],
                                    start=(si == 0),
                                    stop=(si == len(sp) - 1),
                                )
                        gate = gates.tile([128, STRIPE], f32)
                        nc.scalar.activation(
                            gate[:, :ln], ps[:, :ln],
                            mybir.ActivationFunctionType.Sigmoid,
                            bias=bias5[:], scale=100.0,
                        )
                        ot = outs.tile([128, STRIPE], f32)
                        q = c0 % K
                        init = 0.0 if s == 0 else prev_out[:, prev_len - 1:prev_len]
                        nc.vector.tensor_tensor_scan(
                            out=ot[:, :ln],
                            data0=mask[:, q:q + ln],
                            data1=gate[:, :ln],
                            initial=init,
                            op0=mybir.AluOpType.max,
                            op1=mybir.AluOpType.mult,
                        )
                        ot16 = outs16.tile([128, STRIPE], bf16)
                        nc.scalar.copy(ot16[:, :ln], ot[:, :ln])
                        if blk == 0:
                            deferred.append(
                                (out_d[di, blk * 128:(blk + 1) * 128,
                                       c0:c0 + ln], ot16[:, :ln]))
                        else:
                            nc.sync.dma_start(
                                out_d[di, blk * 128:(blk + 1) * 128,
                                      c0:c0 + ln],
                                ot16[:, :ln],
                            )
                        prev_out, prev_len = ot, ln

    nc.compile()
    return nc


_NC = None


def _get_nc():
    global _NC
    if _NC is None:
        _NC = build_program()
    return _NC


def kernel(score, score_idx):
    score = np.ascontiguousarray(np.asarray(score, dtype=np.float32))
    idx = np.ascontiguousarray(np.asarray(score_idx).astype(np.int32))
    assert score.shape == (POOL,) and idx.shape == (N_DOCS, DOC_LEN)

    w_fwd, w_bwd, ident, smask = _build_consts()
    nc = _get_nc()

    in_maps = []
    for c in range(N_CORES):
        in_maps.append({
            "score": score.reshape(POOL, 1),
            "idx": idx[c * DOCS_PER_CORE:(c + 1) * DOCS_PER_CORE],
            "w_fwd": w_fwd,
            "w_bwd": w_bwd,
            "ident": ident,
            "smask": smask,
        })
    res = run_bass_kernel_spmd(nc, in_maps, core_ids=list(range(N_CORES)))
    shards = [np.asarray(r["out"]).astype(np.float32).reshape(2, DOCS_PER_CORE, L, K)
              for r in res.results]
    return np.concatenate(shards, axis=1)


if __name__ == "__main__":
    rng = np.random.default_rng(0)
    score = rng.standard_normal(POOL).astype(np.float32)
    idx = rng.integers(0, POOL, size=(N_DOCS, DOC_LEN)).astype(np.int32)
    out = kernel(score, idx)
    print(out.shape, out.dtype, float(out[0, 0, :4, :4].sum()))

